# revision 53
# baseline (speedup 1.0000x reference)
"""Trainium2 Bass kernel for nn_CustomLSTM: B=32, S=512, D_in=512, D_h=1024, D_out=512.

Strategy (v5): 8-way tensor-parallel over the hidden/gate dim. Core c owns 128
h-dims (block c) and the 4x128 = 512 gate columns that produce them (order
f|i|g|o). Per step:
  - PE: gate preacts split into psum halves A (f,i,g: 384 cols) and B (o: 128
    cols) in SEPARATE psum banks, so ACT can start on A while the PE still
    streams B (no bank collision, chain starts ~0.4us earlier).
  - ACT: sigmoid(f,i) + tanh(g) on A, sigmoid(o) on B -> actsb bf16.
  - DVE: c' = f*c + i*g ; h = o*tanh(c') in batch-major [32,128] all-bf16
    (2x DVE mode, no gate transposes).
  - PE: ONE transpose hsb[32,128] -> h^T [128,32] (psum); ACT copies to hst
    (frees the DVE; ACT is idle there anyway).
  - h^T pushed to all 8 cores' gath slot `pid` with ONE remote_dma_broadcast
    (slot + arrival sem chosen by a per-rank If-chain; per-SENDER x PARITY
    arrival sems s_harr[k][r%2] give the PE a per-slot wait, so it consumes
    tiles one at a time as the packets land -- broadcast flight hides behind
    the staggered matmul stream).
  - Next step's bias/x matmuls are issued right after the slot loop and the
    FC matmuls (out dims [64c,64c+64), all t) after them: PE filler during
    the ACT/DVE/broadcast window (also keeps the PE HAM-warm). The final
    h-transpose sits last, so its completion (via s_T -> ACT copy -> s_hrdy
    -> round-t send) fences this parity's reads against round t+1.
Results stream to DRAM in 8-step chunks.
"""

import os
import sys

if "/opt/trn_rl_repo" not in sys.path:
    sys.path.insert(0, "/opt/trn_rl_repo")

import numpy as np

B, S, DIN, DH, DOUT = 32, 512, 512, 1024, 512
NCORES = 8
LOCH = DH // NCORES          # 128 h-dims per core
LOCG = 4 * LOCH              # 512 gate cols per core (f|i|g|o)
LOCA = 3 * LOCH              # A-half: f,i,g (384)
LOCB = LOCH                  # B-half: o (128)
LOCO = DOUT // NCORES        # 64 fc out-dims per core
KX = DIN // 128              # 4 x k-tiles
KH = 8                       # 8 h slot tiles
HB = B // 2                  # f32 cols per bf16 h^T slice (16)
XT_RING = 8                  # xt prefetch ring depth (steps)
XT_AHEAD = 6                 # prefetch distance
FC_RING = 8                  # fc out staging ring (steps)
SENT_PER_ROUND = 16          # local_sem incs per round (1 broadcast)

# Ablation flags for perf experiments only (default off = full kernel).
ABL_NOBCAST = bool(os.environ.get("LSTM_ABL_NOBCAST"))
ABL_NOFC = bool(os.environ.get("LSTM_ABL_NOFC"))
ABL_NOX = bool(os.environ.get("LSTM_ABL_NOX"))

_cache = {}


def _build_nc(T):
    """Build the SPMD bass program for a T-step LSTM (T divisible by 8)."""
    from concourse import bass
    import concourse.mybir as mybir

    assert T % FC_RING == 0
    dt = mybir.dt
    f32 = dt.float32
    f32r = dt.float32r
    bf16 = dt.bfloat16
    AF = mybir.ActivationFunctionType

    nc = bass.Bass(target_bir_lowering=False, num_devices=NCORES)
    nc.has_collectives = True

    # ---------------- I/O ----------------
    xT = nc.dram_tensor("xT", [DIN, T, B], f32, kind="ExternalInput")
    wxin = nc.dram_tensor("wxin", [DIN, LOCG], f32, kind="ExternalInput")
    whin = nc.dram_tensor("whin", [DH, LOCG], bf16, kind="ExternalInput")
    wfcin = nc.dram_tensor("wfcin", [DH, LOCO], bf16, kind="ExternalInput")
    bin_ = nc.dram_tensor("bin", [1, LOCG], f32, kind="ExternalInput")
    bfcin = nc.dram_tensor("bfcin", [1, LOCO], f32, kind="ExternalInput")
    onesin = nc.dram_tensor("onesin", [1, 256], f32, kind="ExternalInput")
    identbin = nc.dram_tensor("identbin", [128, 128], bf16, kind="ExternalInput")
    outF = nc.dram_tensor("outF", [B, T * LOCO], f32, kind="ExternalOutput")

    # ---------------- semaphores ----------------
    s_ld = nc.alloc_semaphore("s_ld")        # prologue dma loads (+16)
    s_xt = nc.alloc_semaphore("s_xt")        # xt prefetch dmas (+16)
    s_xtg = nc.alloc_semaphore("s_xtg")      # xt groups confirmed (+1)
    s_mmA = nc.alloc_semaphore("s_mmA")      # PE last A-half gate-MM (+1/step)
    s_mm = nc.alloc_semaphore("s_mm")        # PE last B-half gate-MM (+1/step)
    s_act = nc.alloc_semaphore("s_act")      # ACT1/2/3 done (+3/step)
    s_cp = nc.alloc_semaphore("s_cp")        # DVE c' written (+1/step)
    s_tc = nc.alloc_semaphore("s_tc")        # ACT tanh(c') written (+1/step)
    s_hb = nc.alloc_semaphore("s_hb")        # DVE h (batch-major) written (+1/step)
    s_T = nc.alloc_semaphore("s_T")          # PE h-transpose done (+1/step)
    s_hrdy = nc.alloc_semaphore("s_hrdy")    # ACT h^T -> hst copy done (+1/step)
    s_dvf = nc.alloc_semaphore("s_dvf")      # DVE writeback fences (+2/step)
    s_fcm = nc.alloc_semaphore("s_fcm")      # PE last FC-MM (+1/step)
    s_fce = nc.alloc_semaphore("s_fce")      # DVE FC evac (+1/step)
    s_fout = nc.alloc_semaphore("s_fout")    # FC out dmas (+16 per chunk)
    s_hfree = nc.alloc_semaphore("s_hfree")  # Pool relay: hst slot reusable (+1)
    s_prep = nc.alloc_semaphore("s_prep")    # Q7 desc-gen committed (+1/prep)
    # Cross-core h^T arrival, PER-SENDER x PARITY: s_harr[k][r%2] is
    # incremented (+2) by core k's round-r broadcast (slot k of the gather).
    # Per-sender counting gives the PE a per-tile wait; the parity split
    # gives the transitive-ordering chain the one round of slack it needs.
    # Cleared ONLY in the tail (after quiescence), never in the prologue.
    s_harr = [
        [nc.alloc_semaphore(f"s_harr{d}_0"), nc.alloc_semaphore(f"s_harr{d}_1")]
        for d in range(NCORES)
    ]
    # local send-complete, parity-split (+16 on s_sent[r%2] per round)
    s_sent = [nc.alloc_semaphore("s_sent0"), nc.alloc_semaphore("s_sent1")]
    pre_sems = [s_ld, s_xt, s_xtg, s_mmA, s_mm, s_act, s_cp, s_tc, s_hb, s_T,
                s_hrdy, s_dvf, s_fcm, s_fce, s_fout, s_hfree, s_prep]
    all_sems = pre_sems + [s for pair in s_harr for s in pair] + s_sent

    def harr_wait(k, t):
        """(sem, value) guaranteeing slot k's round t-1 (h_t) has landed."""
        return s_harr[k][(t - 1) % 2], 2 * ((t + 1) // 2)

    # ---------------- on-chip tensors ----------------
    ctx_tensors = []

    def sbuf(name, shape, dtype=f32):
        cm = nc.sbuf_tensor(name, shape, dtype)
        t = cm.__enter__()
        ctx_tensors.append(cm)
        return t

    def psum(name, shape, dtype=f32):
        cm = nc.psum_tensor(name, shape, dtype)
        t = cm.__enter__()
        ctx_tensors.append(cm)
        return t

    wx = sbuf("wx", [128, KX * LOCG])          # x-weights, tile d at cols d*512
    wh = sbuf("wh", [128, KH * LOCG], bf16)    # h-weights, tile m at m*512
    wfc = sbuf("wfc", [128, KH * LOCO], bf16)  # fc weights, tile m at m*64
    xt = sbuf("xt", [128, XT_RING * KX * B])   # x_t^T ring: step r at cols r*128
    # gathered h^T: bf16 payload viewed through an f32 tensor (the broadcast
    # ISA encoding only round-trips natural f32 APs). 2 parities x 8 slots.
    gath = sbuf("gath", [128, 2 * 128])
    hst = sbuf("hst", [128, 2 * HB])           # h^T staging (bf16-as-f32)
    actsb = sbuf("actsb", [32, 2 * LOCG], bf16)  # activated gates f|i|g|o
    csb = sbuf("csb", [32, 2 * LOCH], bf16)    # c state, 2 parities
    t1b = sbuf("t1b", [32, LOCH], bf16)
    t2b = sbuf("t2b", [32, LOCH], bf16)
    tcb = sbuf("tcb", [32, LOCH], bf16)        # tanh(c')
    hsb = sbuf("hsb", [32, 2 * LOCH], bf16)    # h batch-major, 2 parities
    fcring = sbuf("fcring", [32, FC_RING * LOCO])  # fc out staging
    bvec = sbuf("bvec", [1, LOCG])
    bfcv = sbuf("bfcv", [1, LOCO])
    onesb = sbuf("onesb", [1, 256])
    identb = sbuf("identb", [128, 128], bf16)

    # psum: 8 banks exactly. pg: per parity, A-half bank + B-half bank.
    pg = psum("pg", [128, 4 * 512])            # [par0A | par0B | par1A | par1B]
    pt = psum("pt", [128, 2 * 512])            # h^T transpose out, 2 banks
    pfc = psum("pfc", [128, 2 * 512])          # fc psum, 2 banks (rows 0:32)

    def pgA(t):
        return pg[0:32, 1024 * (t % 2):1024 * (t % 2) + LOCA]

    def pgB(t):
        return pg[0:32, 1024 * (t % 2) + 512:1024 * (t % 2) + 512 + LOCB]

    def r(ap):
        return ap.bitcast(f32r)

    # ================= Block P0: clear local sems =================
    with nc.Block() as blk:
        @blk.gpsimd
        def _(gp):
            for sm in pre_sems:
                gp.sem_clear(sm)

    # ================= Block P1: load weights / init state =================
    with nc.Block() as blk:
        @blk.sync
        def _(sp):
            n = 0
            for d in range(KX):
                sp.dma_start(r(wx[:, 512 * d:512 * (d + 1)]),
                             r(wxin[128 * d:128 * (d + 1), :])).then_inc(s_ld, 16)
                n += 1
            for m in range(KH):
                sp.dma_start(wh[:, 512 * m:512 * (m + 1)],
                             whin[128 * m:128 * (m + 1), :]).then_inc(s_ld, 16)
                n += 1
            for m in range(KH):
                sp.dma_start(wfc[:, 64 * m:64 * (m + 1)],
                             wfcin[128 * m:128 * (m + 1), :]).then_inc(s_ld, 16)
                n += 1
            P = min(XT_AHEAD, T)
            if not ABL_NOX:
                for t0 in range(P):
                    for d in range(KX):
                        sp.dma_start(
                            r(xt[:, 128 * t0 + 32 * d:128 * t0 + 32 * (d + 1)]),
                            r(xT[128 * d:128 * (d + 1), t0, :])).then_inc(s_xt, 16)
                sp.wait_ge(s_xt, 16 * KX * P)
                sp.sem_inc(s_xtg, P)
            sp.dma_start(r(bvec[0:1, :]), r(bin_[0:1, :])).then_inc(s_ld, 16); n += 1
            sp.dma_start(r(bfcv[0:1, :]), r(bfcin[0:1, :])).then_inc(s_ld, 16); n += 1
            sp.dma_start(r(onesb[0:1, :]), r(onesin[0:1, :])).then_inc(s_ld, 16); n += 1
            sp.dma_start(identb[:, :], identbin[:, :]).then_inc(s_ld, 16); n += 1
            sp.wait_ge(s_ld, 16 * n)

        @blk.vector
        def _(ve):
            ve.memset(csb[0:32, 0:LOCH], 0.0)    # c_0 = 0 (parity 0)

    # ================= Block M: the recurrence =================
    with nc.Block() as blk:
        @blk.tensor
        def _(te):
            def bias_x(t):
                """bias + x part of step t, split into the A/B psum halves
                (independent of h: issued one step early as chain filler)."""
                if t >= 2:
                    te.wait_ge(s_act, 3 * t - 3)               # pg[t%2] free
                if not ABL_NOX:
                    te.wait_ge(s_xtg, t + 1)                   # xt(t) loaded
                xs = 128 * (t % XT_RING)
                for half, pb, c0, cn in ((0, pgA(t), 0, LOCA),
                                         (1, pgB(t), LOCA, LOCB)):
                    sm = s_mmA if half == 0 else s_mm
                    mm = te.matmul(pb, r(onesb[0:1, 0:32]),
                                   r(bvec[0:1, c0:c0 + cn]),
                                   start=True, stop=(t == 0 and ABL_NOX))
                    if t == 0 and ABL_NOX:
                        mm.then_inc(sm, 1)
                    for d in range(KX if not ABL_NOX else 0):
                        mm = te.matmul(
                            pb, r(xt[:, xs + 32 * d:xs + 32 * (d + 1)]),
                            r(wx[:, 512 * d + c0:512 * d + c0 + cn]),
                            start=False, stop=(t == 0 and d == KX - 1))
                        if t == 0 and d == KX - 1:
                            # h_0 = 0: step 0 has no h-MMs; close groups here.
                            mm.then_inc(sm, 1)

            bias_x(0)
            for t in range(T):
                pi = t % 2
                gb = 128 * pi
                # --- slot loops: per-slot waits, consuming tiles one at a
                # time as each sender's packet lands (slot i = core i's tile).
                # A-half (f,i,g) first so ACT starts while B still streams.
                if t >= 1:
                    for half, pb, c0, cn in ((0, pgA(t), 0, LOCA),
                                             (1, pgB(t), LOCA, LOCB)):
                        sm = s_mmA if half == 0 else s_mm
                        for i in range(KH):
                            if half == 0 and not ABL_NOBCAST:
                                te.wait_ge(*harr_wait(i, t))   # round t-1 landed
                            mm = te.matmul(
                                pb,
                                gath[:, gb + 16 * i:gb + 16 * (i + 1)].bitcast(bf16),
                                wh[:, 512 * i + c0:512 * i + c0 + cn],
                                start=False, stop=(i == KH - 1))
                            if i == KH - 1:
                                mm.then_inc(sm, 1)
                # --- next step's bias+x (fills the early chain window)
                if t + 1 < T:
                    bias_x(t + 1)
                # --- FC for output index j = t-1 (reads the same gathered
                # parity as the h-MMs above; PE filler during the chain /
                # broadcast window, keeps HAM warm)
                if t >= 1 and not ABL_NOFC:
                    j = t - 1
                    pfcb = pfc[0:32, 512 * (j % 2):512 * (j % 2) + LOCO]
                    if j >= 2:
                        te.wait_ge(s_fce, j - 1)               # pfc[j%2] free
                    te.matmul(pfcb, r(onesb[0:1, 0:32]), r(bfcv[0:1, :]),
                              start=True, stop=False)
                    for i in range(KH):
                        mm = te.matmul(
                            pfcb,
                            gath[:, gb + 16 * i:gb + 16 * (i + 1)].bitcast(bf16),
                            wfc[:, 64 * i:64 * (i + 1)],
                            start=False, stop=(i == KH - 1))
                        if i == KH - 1:
                            mm.then_inc(s_fcm, 1)
                # --- transpose of h(t): hsb[t%2] -> pt[t%2]. Last in the
                # iteration: its completion (via s_T -> ACT copy -> s_hrdy ->
                # round-t send) also fences this iteration's parity reads
                # (h-MMs + FC) against round t+1's overwrite.
                te.wait_ge(s_hb, t + 1)                        # DVE h(t) done
                te.matmul(pt[:, 512 * pi:512 * pi + 16].bitcast(bf16),
                          hsb[0:32, 128 * pi:128 * (pi + 1)],
                          identb[0:32, 0:32],
                          is_transpose=True, start=True, stop=True,
                          skip_group_check=True).then_inc(s_T, 1)
            # epilogue: FC for output index T-1 (h_T, gathered by round T-1)
            if not ABL_NOFC:
                j = T - 1
                gb = 128 * (T % 2)
                pfcb = pfc[0:32, 512 * (j % 2):512 * (j % 2) + LOCO]
                te.wait_ge(s_fce, j - 1)
                te.matmul(pfcb, r(onesb[0:1, 0:32]), r(bfcv[0:1, :]),
                          start=True, stop=False)
                for i in range(KH):
                    if not ABL_NOBCAST:
                        te.wait_ge(*harr_wait(i, T))
                    mm = te.matmul(
                        pfcb,
                        gath[:, gb + 16 * i:gb + 16 * (i + 1)].bitcast(bf16),
                        wfc[:, 64 * i:64 * (i + 1)],
                        start=False, stop=(i == KH - 1))
                    if i == KH - 1:
                        mm.then_inc(s_fcm, 1)

        @blk.scalar
        def _(ac):
            for t in range(T):
                pi = t % 2
                po = (t + 1) % 2
                ab = 512 * pi
                pga = 1024 * pi
                ac.wait_ge(s_mmA, t + 1)
                if t >= 2:
                    ac.wait_ge(s_hb, t - 1)                    # actsb[pi] free
                ac.activation(actsb[0:32, ab:ab + 256],
                              pg[0:32, pga:pga + 256],
                              AF.Sigmoid).then_inc(s_act, 1)   # f, i
                ac.activation(actsb[0:32, ab + 256:ab + 384],
                              pg[0:32, pga + 256:pga + 384],
                              AF.Tanh).then_inc(s_act, 1)      # g
                ac.wait_ge(s_mm, t + 1)
                ac.activation(actsb[0:32, ab + 384:ab + 512],
                              pg[0:32, pga + 512:pga + 640],
                              AF.Sigmoid).then_inc(s_act, 1)   # o
                # tanh(c_{t+1})
                ac.wait_ge(s_cp, t + 1)
                if t >= 1:
                    ac.wait_ge(s_hb, t)                        # tcb free
                ac.activation(tcb[0:32, :], csb[0:32, 128 * po:128 * po + 128],
                              AF.Tanh).then_inc(s_tc, 1)
                # h^T: pt[pi] -> hst[pi] (ACT is idle here; frees the DVE)
                ac.wait_ge(s_T, t + 1)
                if t >= 2 and not ABL_NOBCAST:
                    ac.wait_ge(s_hfree, t - 1)                 # hst[pi] flushed
                ac.copy(hst[:, HB * pi:HB * (pi + 1)].bitcast(bf16),
                        pt[:, 512 * pi:512 * pi + 16].bitcast(bf16)
                        ).then_inc(s_hrdy, 1)

        @blk.vector
        def _(ve):
            for t in range(T):
                pi = t % 2
                po = (t + 1) % 2
                ab = 512 * pi
                # c' = f*c + i*g  (batch-major [32,128], all bf16 -> 2x mode)
                ve.wait_ge(s_act, 3 * t + 1)
                ve.tensor_mul(t1b[0:32, :], actsb[0:32, ab:ab + 128],
                              csb[0:32, 128 * pi:128 * pi + 128]).then_inc(s_dvf, 1)
                ve.wait_ge(s_act, 3 * t + 2)
                ve.tensor_mul(t2b[0:32, :], actsb[0:32, ab + 128:ab + 256],
                              actsb[0:32, ab + 256:ab + 384]).then_inc(s_dvf, 1)
                if t >= 2:
                    ve.wait_ge(s_tc, t - 1)                    # csb[po] free
                ve.wait_ge(s_dvf, 2 * t + 2)                   # t1/t2 writeback fence
                ve.tensor_add(csb[0:32, 128 * po:128 * po + 128],
                              t1b[0:32, :], t2b[0:32, :]).then_inc(s_cp, 1)
                # h = o * tanh(c')
                ve.wait_ge(s_act, 3 * t + 3)
                ve.wait_ge(s_tc, t + 1)
                if t >= 2:
                    ve.wait_ge(s_T, t - 1)                     # hsb[pi] free
                ve.tensor_mul(hsb[0:32, 128 * pi:128 * (pi + 1)],
                              actsb[0:32, ab + 384:ab + 512],
                              tcb[0:32, :]).then_inc(s_hb, 1)
                # FC evac psum -> staging ring (output index j = t-1)
                if t >= 1 and not ABL_NOFC:
                    j = t - 1
                    ve.wait_ge(s_fcm, j + 1)
                    if j >= FC_RING:
                        ve.wait_ge(s_fout, 16 * (j // FC_RING))  # ring slot free
                    ve.tensor_copy(fcring[0:32, LOCO * (j % FC_RING):
                                           LOCO * (j % FC_RING + 1)],
                                   pfc[0:32, 512 * (j % 2):
                                       512 * (j % 2) + LOCO]).then_inc(s_fce, 1)
            # epilogue evac for output index T-1
            if not ABL_NOFC:
                j = T - 1
                ve.wait_ge(s_fcm, j + 1)
                ve.wait_ge(s_fout, 16 * (j // FC_RING))
                ve.tensor_copy(fcring[0:32, LOCO * (j % FC_RING):
                                       LOCO * (j % FC_RING + 1)],
                               pfc[0:32, 512 * (j % 2):
                                   512 * (j % 2) + LOCO]).then_inc(s_fce, 1)

        @blk.sync
        def _(sp):
            nxt = 16 * KX * min(XT_AHEAD, T)
            for t in range(T):
                tf = t + XT_AHEAD
                if tf < T and not ABL_NOX:
                    if t >= 2:
                        sp.wait_ge(s_mm, t - 1)                # ring slot free
                    xs = 128 * (tf % XT_RING)
                    for d in range(KX):
                        sp.dma_start(r(xt[:, xs + 32 * d:xs + 32 * (d + 1)]),
                                     r(xT[128 * d:128 * (d + 1), tf, :])).then_inc(
                                         s_xt, 16)
                    nxt += 16 * KX
                    sp.wait_ge(s_xt, nxt)
                    sp.sem_inc(s_xtg, 1)
                # output chunk q covers output indices [8q, 8q+8)
                if t % FC_RING == 0 and t >= FC_RING and not ABL_NOFC:
                    q = t // FC_RING - 1
                    sp.wait_ge(s_fce, FC_RING * (q + 1))
                    sp.dma_start(outF[:, LOCO * FC_RING * q:
                                      LOCO * FC_RING * (q + 1)],
                                 fcring[0:32, :]).then_inc(s_fout, 16)
                    sp.wait_ge(s_fout, 16 * (q + 1))
            # final output chunk (indices [T-8, T))
            if not ABL_NOFC:
                q = T // FC_RING - 1
                sp.wait_ge(s_fce, T)
                sp.dma_start(outF[:, LOCO * FC_RING * q:LOCO * FC_RING * (q + 1)],
                             fcring[0:32, :]).then_inc(s_fout, 16)
                sp.wait_ge(s_fout, 16 * (q + 1))

        @blk.gpsimd
        def _(gp):
            if ABL_NOBCAST:
                return
            from concourse import library_config
            gp.load_library(library_config.remote_dma)
            pidreg = gp.alloc_register("pidreg")
            gp.reg_load(pidreg, nc.partition_id_tensor[0:1, 0:1])
            RD = [(0, k) for k in range(NCORES)]

            def prep(round_t):
                """Queue the round-t broadcast: hst parity round_t%2 into
                slot `pid` of gath parity (round_t+1)%2 on all 8 cores (slot
                and arrival sem chosen by sender rank via the If-chain)."""
                ps = HB * (round_t % 2)
                gbase = 128 * ((round_t + 1) % 2)
                for k in range(NCORES):
                    with gp.If_eq(pidreg, k):
                        gp.remote_dma_broadcast(
                            gath[:, gbase + 16 * k:gbase + 16 * (k + 1)],
                            hst[:, ps:ps + HB],
                            s_harr[k][round_t % 2], s_sent[round_t % 2],
                            rdests=RD).then_inc(s_prep, 1)

            for t in range(T):
                # Mirror the PE's arrival waits (always already satisfied
                # here, transitively through this core's own PE step t) so
                # the round-t send carries "every peer consumed round t-2"
                # explicitly on the sending engine. Must precede the desc-gen
                # (ordering binds at prep time).
                if t >= 1:
                    for k in range(NCORES):
                        gp.wait_ge(*harr_wait(k, t))
                prep(t)
                # Q7 desc-gen must commit to the SBUF ring before the
                # trigger's doorbell write (HW race; the sim models prep
                # synchronously and can't catch it).
                gp.wait_ge(s_prep, t + 1)
                gp.wait_ge(s_hrdy, t + 1)
                gp.trigger_dma(1)
                # Flush-wait for the PREVIOUS round (its packets left during
                # step t's compute, so this is free), then relay to the ACT
                # (hst[(t-1)%2] reusable). Keeps round t's flush latency off
                # the Pool's critical path.
                if t >= 1:
                    gp.wait_ge(s_sent[(t - 1) % 2],
                               SENT_PER_ROUND * ((t + 1) // 2))
                    gp.sem_inc(s_hfree, 1)
            # quiescence: all sends flushed, all peers' packets landed
            gp.wait_ge(s_sent[(T - 1) % 2], SENT_PER_ROUND * ((T + 1) // 2))
            gp.sem_inc(s_hfree, 1)
            for k in range(NCORES):
                gp.wait_ge(s_harr[k][0], 2 * ((T + 1) // 2))
                gp.wait_ge(s_harr[k][1], 2 * (T // 2))

    # ================= Block E: final cleanup =================
    with nc.Block() as blk:
        @blk.gpsimd
        def _(gp):
            for sm in all_sems:
                gp.sem_clear(sm)

    for cm in reversed(ctx_tensors):
        cm.__exit__(None, None, None)
    mybir.codegen_inst_isa_subclasses(nc)
    return nc


def _prep_in_maps(inputs, T=S):
    """Host-side sharding: per-core input dicts (gate order f|i|g|o)."""
    x = np.ascontiguousarray(np.asarray(inputs["x"], np.float32)[:, :T, :])
    W_f = np.asarray(inputs["W_f"], np.float32)
    W_i = np.asarray(inputs["W_i"], np.float32)
    W_g = np.asarray(inputs["W_g"], np.float32)
    W_o = np.asarray(inputs["W_o"], np.float32)
    b_f = np.asarray(inputs["b_f"], np.float32)
    b_i = np.asarray(inputs["b_i"], np.float32)
    b_g = np.asarray(inputs["b_g"], np.float32)
    b_o = np.asarray(inputs["b_o"], np.float32)
    W_fc = np.ascontiguousarray(np.asarray(inputs["W_fc"], np.float32))
    b_fc = np.asarray(inputs["b_fc"], np.float32)

    import ml_dtypes
    bf16 = ml_dtypes.bfloat16
    xT = np.ascontiguousarray(x.transpose(2, 1, 0))  # [DIN, T, B]
    ones = np.ones((1, 256), np.float32)
    eyeb = np.eye(128, dtype=bf16)
    in_maps = []
    for c in range(NCORES):
        sl = slice(LOCH * c, LOCH * (c + 1))
        so = slice(LOCO * c, LOCO * (c + 1))
        Wcat = np.concatenate(
            [W_f[:, sl], W_i[:, sl], W_g[:, sl], W_o[:, sl]], axis=1)  # f|i|g|o
        Wx_c = np.ascontiguousarray(Wcat[:DIN])
        Wh_c = Wcat[DIN:]                                  # [DH, LOCG]
        Wfc_c = W_fc[:, so]
        b_c = np.concatenate([b_f[sl], b_i[sl], b_g[sl], b_o[sl]])[None, :]
        in_maps.append({
            "xT": xT,
            "wxin": Wx_c,
            "whin": np.ascontiguousarray(Wh_c).astype(bf16),
            "wfcin": np.ascontiguousarray(Wfc_c).astype(bf16),
            "bin": np.ascontiguousarray(b_c),
            "bfcin": np.ascontiguousarray(b_fc[None, so]),
            "onesin": ones,
            "identbin": eyeb,
        })
    return in_maps


def _assemble(results, T=S):
    out = np.empty((B, T, DOUT), np.float32)
    for c in range(NCORES):
        blk = np.asarray(results[c]["outF"], np.float32).reshape(B, T, LOCO)
        out[:, :, LOCO * c:LOCO * (c + 1)] = blk
    return out


def get_nc(T=S):
    if T not in _cache:
        _cache[T] = _build_nc(T)
    return _cache[T]


def kernel(**inputs):
    from concourse import bass_utils
    nc = get_nc(S)
    in_maps = _prep_in_maps(inputs, S)
    res = bass_utils.run_bass_kernel_spmd(nc, in_maps, core_ids=list(range(NCORES)))
    return _assemble(res.results, S)


# revision 62
# speedup vs baseline: 1.0476x; 1.0476x over previous
"""Trainium2 Bass kernel for nn_CustomLSTM: B=32, S=512, D_in=512, D_h=1024, D_out=512.

Strategy (v5): 8-way tensor-parallel over the hidden/gate dim. Core c owns 128
h-dims (block c) and the 4x128 = 512 gate columns that produce them (order
f|i|g|o). Per step:
  - PE: gate preacts split into psum halves A (f,i,g: 384 cols) and B (o: 128
    cols) in SEPARATE psum banks, so ACT can start on A while the PE still
    streams B (no bank collision, chain starts ~0.4us earlier).
  - ACT: sigmoid(f,i) + tanh(g) on A, sigmoid(o) on B -> actsb bf16.
  - DVE: c' = f*c + i*g ; h = o*tanh(c') in batch-major [32,128] all-bf16
    (2x DVE mode, no gate transposes).
  - PE: ONE transpose hsb[32,128] -> h^T [128,32] (psum); ACT copies to hst
    (frees the DVE; ACT is idle there anyway).
  - h^T pushed to all 8 cores' gath slot `pid` with ONE remote_dma_broadcast
    (slot + arrival sem chosen by a per-rank If-chain; per-SENDER x PARITY
    arrival sems s_harr[k][r%2] give the PE a per-slot wait, so it consumes
    tiles one at a time as the packets land -- broadcast flight hides behind
    the staggered matmul stream).
  - Next step's bias/x matmuls are issued right after the slot loop and the
    FC matmuls (out dims [64c,64c+64), all t) after them: PE filler during
    the ACT/DVE/broadcast window (also keeps the PE HAM-warm). The final
    h-transpose sits last, so its completion (via s_T -> ACT copy -> s_hrdy
    -> round-t send) fences this parity's reads against round t+1.
Results stream to DRAM in 8-step chunks.
"""

import os
import sys

if "/opt/trn_rl_repo" not in sys.path:
    sys.path.insert(0, "/opt/trn_rl_repo")

import numpy as np

B, S, DIN, DH, DOUT = 32, 512, 512, 1024, 512
NCORES = 8
LOCH = DH // NCORES          # 128 h-dims per core
LOCG = 4 * LOCH              # 512 gate cols per core (f|i|g|o)
LOCA = 3 * LOCH              # A-half: f,i,g (384)
LOCB = LOCH                  # B-half: o (128)
LOCO = DOUT // NCORES        # 64 fc out-dims per core
KX = DIN // 128              # 4 x k-tiles
KH = 8                       # 8 h slot tiles
HB = B // 2                  # f32 cols per bf16 h^T slice (16)
XT_RING = 8                  # xt prefetch ring depth (steps)
XT_AHEAD = 6                 # prefetch distance
FC_RING = 8                  # fc out staging ring (steps)
SENT_PER_ROUND = 16          # local_sem incs per round (1 broadcast)

# Ablation flags for perf experiments only (default off = full kernel).
ABL_NOBCAST = bool(os.environ.get("LSTM_ABL_NOBCAST"))
ABL_NOFC = bool(os.environ.get("LSTM_ABL_NOFC"))
ABL_NOX = bool(os.environ.get("LSTM_ABL_NOX"))

_cache = {}


def _build_nc(T):
    """Build the SPMD bass program for a T-step LSTM (T divisible by 8)."""
    from concourse import bass
    import concourse.mybir as mybir

    assert T % FC_RING == 0
    dt = mybir.dt
    f32 = dt.float32
    f32r = dt.float32r
    bf16 = dt.bfloat16
    AF = mybir.ActivationFunctionType

    nc = bass.Bass(target_bir_lowering=False, num_devices=NCORES)
    nc.has_collectives = True

    # ---------------- I/O ----------------
    xT = nc.dram_tensor("xT", [DIN, T, B], f32, kind="ExternalInput")
    wxin = nc.dram_tensor("wxin", [DIN, LOCG], f32, kind="ExternalInput")
    whin = nc.dram_tensor("whin", [DH, LOCG], bf16, kind="ExternalInput")
    wfcin = nc.dram_tensor("wfcin", [DH, LOCO], bf16, kind="ExternalInput")
    bin_ = nc.dram_tensor("bin", [1, LOCG], f32, kind="ExternalInput")
    bfcin = nc.dram_tensor("bfcin", [1, LOCO], f32, kind="ExternalInput")
    onesin = nc.dram_tensor("onesin", [1, 256], f32, kind="ExternalInput")
    identbin = nc.dram_tensor("identbin", [128, 128], bf16, kind="ExternalInput")
    outF = nc.dram_tensor("outF", [B, T * LOCO], f32, kind="ExternalOutput")

    # ---------------- semaphores ----------------
    s_ld = nc.alloc_semaphore("s_ld")        # prologue dma loads (+16)
    s_xt = nc.alloc_semaphore("s_xt")        # xt prefetch dmas (+16)
    s_xtg = nc.alloc_semaphore("s_xtg")      # xt groups confirmed (+1)
    s_mm = nc.alloc_semaphore("s_mm")        # PE last gate-MM (+1/step)
    s_act = nc.alloc_semaphore("s_act")      # ACT sigmoid/tanh done (+2/step)
    s_cp = nc.alloc_semaphore("s_cp")        # DVE c' written (+1/step)
    s_tc = nc.alloc_semaphore("s_tc")        # ACT tanh(c') written (+1/step)
    s_hb = nc.alloc_semaphore("s_hb")        # DVE h (batch-major) written (+1/step)
    s_T = nc.alloc_semaphore("s_T")          # PE h-transpose done (+1/step)
    s_hrdy = nc.alloc_semaphore("s_hrdy")    # ACT h^T -> hst copy done (+1/step)
    s_dvf = nc.alloc_semaphore("s_dvf")      # DVE writeback fences (+2/step)
    s_fcm = nc.alloc_semaphore("s_fcm")      # PE last FC-MM (+1/step)
    s_fce = nc.alloc_semaphore("s_fce")      # DVE FC evac (+1/step)
    s_fout = nc.alloc_semaphore("s_fout")    # FC out dmas (+16 per chunk)
    s_hfree = nc.alloc_semaphore("s_hfree")  # Pool relay: hst slot reusable (+1)
    s_prep = nc.alloc_semaphore("s_prep")    # Q7 desc-gen committed (+1/prep)
    # Cross-core h^T arrival, PER-SENDER x PARITY: s_harr[k][r%2] is
    # incremented (+2) by core k's round-r broadcast (slot k of the gather).
    # Per-sender counting gives the PE a per-tile wait; the parity split
    # gives the transitive-ordering chain the one round of slack it needs.
    # Cleared ONLY in the tail (after quiescence), never in the prologue.
    s_harr = [
        [nc.alloc_semaphore(f"s_harr{d}_0"), nc.alloc_semaphore(f"s_harr{d}_1")]
        for d in range(NCORES)
    ]
    # local send-complete, parity-split (+16 on s_sent[r%2] per round)
    s_sent = [nc.alloc_semaphore("s_sent0"), nc.alloc_semaphore("s_sent1")]
    pre_sems = [s_ld, s_xt, s_xtg, s_mm, s_act, s_cp, s_tc, s_hb, s_T,
                s_hrdy, s_dvf, s_fcm, s_fce, s_fout, s_hfree, s_prep]
    all_sems = pre_sems + [s for pair in s_harr for s in pair] + s_sent

    def harr_wait(k, t):
        """(sem, value) guaranteeing slot k's round t-1 (h_t) has landed."""
        return s_harr[k][(t - 1) % 2], 2 * ((t + 1) // 2)

    # ---------------- on-chip tensors ----------------
    ctx_tensors = []

    def sbuf(name, shape, dtype=f32):
        cm = nc.sbuf_tensor(name, shape, dtype)
        t = cm.__enter__()
        ctx_tensors.append(cm)
        return t

    def psum(name, shape, dtype=f32):
        cm = nc.psum_tensor(name, shape, dtype)
        t = cm.__enter__()
        ctx_tensors.append(cm)
        return t

    wx = sbuf("wx", [128, KX * LOCG])          # x-weights, tile d at cols d*512
    wh = sbuf("wh", [128, KH * LOCG], bf16)    # h-weights, tile m at m*512
    wfc = sbuf("wfc", [128, KH * LOCO], bf16)  # fc weights, tile m at m*64
    xt = sbuf("xt", [128, XT_RING * KX * B])   # x_t^T ring: step r at cols r*128
    # gathered h^T: bf16 payload viewed through an f32 tensor (the broadcast
    # ISA encoding only round-trips natural f32 APs). 2 parities x 8 slots.
    gath = sbuf("gath", [128, 2 * 128])
    hst = sbuf("hst", [128, 2 * HB])           # h^T staging (bf16-as-f32)
    actsb = sbuf("actsb", [32, 2 * LOCG], bf16)  # activated gates f|i|g|o
    csb = sbuf("csb", [32, 2 * LOCH], bf16)    # c state, 2 parities
    t1b = sbuf("t1b", [32, LOCH], bf16)
    t2b = sbuf("t2b", [32, LOCH], bf16)
    tcb = sbuf("tcb", [32, LOCH], bf16)        # tanh(c')
    hsb = sbuf("hsb", [32, 2 * LOCH], bf16)    # h batch-major, 2 parities
    fcring = sbuf("fcring", [32, FC_RING * LOCO])  # fc out staging
    bvec = sbuf("bvec", [1, LOCG])
    bfcv = sbuf("bfcv", [1, LOCO])
    onesb = sbuf("onesb", [1, 256])
    identb = sbuf("identb", [128, 128], bf16)

    pg = psum("pg", [128, 2 * 512])            # gate psum, 2 banks (rows 0:32)
    pt = psum("pt", [128, 2 * 512])            # h^T transpose out, 2 banks
    pfc = psum("pfc", [128, 2 * 512])          # fc psum, 2 banks (rows 0:32)

    def r(ap):
        return ap.bitcast(f32r)

    # ================= Block P0: clear local sems =================
    with nc.Block() as blk:
        @blk.gpsimd
        def _(gp):
            for sm in pre_sems:
                gp.sem_clear(sm)

    # ================= Block P1: load weights / init state =================
    with nc.Block() as blk:
        @blk.sync
        def _(sp):
            n = 0
            for d in range(KX):
                sp.dma_start(r(wx[:, 512 * d:512 * (d + 1)]),
                             r(wxin[128 * d:128 * (d + 1), :])).then_inc(s_ld, 16)
                n += 1
            for m in range(KH):
                sp.dma_start(wh[:, 512 * m:512 * (m + 1)],
                             whin[128 * m:128 * (m + 1), :]).then_inc(s_ld, 16)
                n += 1
            for m in range(KH):
                sp.dma_start(wfc[:, 64 * m:64 * (m + 1)],
                             wfcin[128 * m:128 * (m + 1), :]).then_inc(s_ld, 16)
                n += 1
            P = min(XT_AHEAD, T)
            if not ABL_NOX:
                for t0 in range(P):
                    for d in range(KX):
                        sp.dma_start(
                            r(xt[:, 128 * t0 + 32 * d:128 * t0 + 32 * (d + 1)]),
                            r(xT[128 * d:128 * (d + 1), t0, :])).then_inc(s_xt, 16)
                sp.wait_ge(s_xt, 16 * KX * P)
                sp.sem_inc(s_xtg, P)
            sp.dma_start(r(bvec[0:1, :]), r(bin_[0:1, :])).then_inc(s_ld, 16); n += 1
            sp.dma_start(r(bfcv[0:1, :]), r(bfcin[0:1, :])).then_inc(s_ld, 16); n += 1
            sp.dma_start(r(onesb[0:1, :]), r(onesin[0:1, :])).then_inc(s_ld, 16); n += 1
            sp.dma_start(identb[:, :], identbin[:, :]).then_inc(s_ld, 16); n += 1
            sp.wait_ge(s_ld, 16 * n)

        @blk.vector
        def _(ve):
            ve.memset(csb[0:32, 0:LOCH], 0.0)    # c_0 = 0 (parity 0)

    # ================= Block M: the recurrence =================
    with nc.Block() as blk:
        @blk.tensor
        def _(te):
            def bias_x(t):
                """bias + x part of step t (independent of h: issued one step
                early as chain filler)."""
                pgb = pg[0:32, 512 * (t % 2):512 * (t % 2 + 1)]
                if t >= 2:
                    te.wait_ge(s_act, 2 * t - 2)               # pg[t%2] free
                if not ABL_NOX:
                    te.wait_ge(s_xtg, t + 1)                   # xt(t) loaded
                xs = 128 * (t % XT_RING)
                mm = te.matmul(pgb, r(onesb[0:1, 0:32]), r(bvec[0:1, :]),
                               start=True, stop=(t == 0 and ABL_NOX))
                if t == 0 and ABL_NOX:
                    mm.then_inc(s_mm, 1)
                for d in range(KX if not ABL_NOX else 0):
                    mm = te.matmul(pgb, r(xt[:, xs + 32 * d:xs + 32 * (d + 1)]),
                                   r(wx[:, 512 * d:512 * (d + 1)]),
                                   start=False, stop=(t == 0 and d == KX - 1))
                    if t == 0 and d == KX - 1:
                        # h_0 = 0: step 0 has no h-MMs; close the group here.
                        mm.then_inc(s_mm, 1)

            bias_x(0)
            for t in range(T):
                pi = t % 2
                gb = 128 * pi
                # --- slot loops: per-slot waits, consuming tiles one at a
                # time as each sender's packet lands (slot i = core i's tile).
                # A-half (f,i,g) first so ACT starts while B still streams.
                if t >= 1:
                    pgb = pg[0:32, 512 * pi:512 * (pi + 1)]
                    for i in range(KH):
                        if not ABL_NOBCAST:
                            te.wait_ge(*harr_wait(i, t))       # round t-1 landed
                        mm = te.matmul(
                            pgb,
                            gath[:, gb + 16 * i:gb + 16 * (i + 1)].bitcast(bf16),
                            wh[:, 512 * i:512 * (i + 1)],
                            start=False, stop=(i == KH - 1))
                        if i == KH - 1:
                            mm.then_inc(s_mm, 1)
                # --- next step's bias+x (fills the early chain window)
                if t + 1 < T:
                    bias_x(t + 1)
                # --- FC for output index j = t-1 (reads the same gathered
                # parity as the h-MMs above; PE filler during the chain /
                # broadcast window, keeps HAM warm)
                if t >= 1 and not ABL_NOFC:
                    j = t - 1
                    pfcb = pfc[0:32, 512 * (j % 2):512 * (j % 2) + LOCO]
                    if j >= 2:
                        te.wait_ge(s_fce, j - 1)               # pfc[j%2] free
                    te.matmul(pfcb, r(onesb[0:1, 0:32]), r(bfcv[0:1, :]),
                              start=True, stop=False)
                    for i in range(KH):
                        mm = te.matmul(
                            pfcb,
                            gath[:, gb + 16 * i:gb + 16 * (i + 1)].bitcast(bf16),
                            wfc[:, 64 * i:64 * (i + 1)],
                            start=False, stop=(i == KH - 1))
                        if i == KH - 1:
                            mm.then_inc(s_fcm, 1)
                # --- transpose of h(t): hsb[t%2] -> pt[t%2]. Last in the
                # iteration: its completion (via s_T -> ACT copy -> s_hrdy ->
                # round-t send) also fences this iteration's parity reads
                # (h-MMs + FC) against round t+1's overwrite.
                te.wait_ge(s_hb, t + 1)                        # DVE h(t) done
                te.matmul(pt[:, 512 * pi:512 * pi + 16].bitcast(bf16),
                          hsb[0:32, 128 * pi:128 * (pi + 1)],
                          identb[0:32, 0:32],
                          is_transpose=True, start=True, stop=True,
                          skip_group_check=True).then_inc(s_T, 1)
            # epilogue: FC for output index T-1 (h_T, gathered by round T-1)
            if not ABL_NOFC:
                j = T - 1
                gb = 128 * (T % 2)
                pfcb = pfc[0:32, 512 * (j % 2):512 * (j % 2) + LOCO]
                te.wait_ge(s_fce, j - 1)
                te.matmul(pfcb, r(onesb[0:1, 0:32]), r(bfcv[0:1, :]),
                          start=True, stop=False)
                for i in range(KH):
                    if not ABL_NOBCAST:
                        te.wait_ge(*harr_wait(i, T))
                    mm = te.matmul(
                        pfcb,
                        gath[:, gb + 16 * i:gb + 16 * (i + 1)].bitcast(bf16),
                        wfc[:, 64 * i:64 * (i + 1)],
                        start=False, stop=(i == KH - 1))
                    if i == KH - 1:
                        mm.then_inc(s_fcm, 1)

        @blk.scalar
        def _(ac):
            for t in range(T):
                pi = t % 2
                po = (t + 1) % 2
                ab = 512 * pi
                ac.wait_ge(s_mm, t + 1)
                if t >= 2:
                    ac.wait_ge(s_hb, t - 1)                    # actsb[pi] free
                ac.activation(actsb[0:32, ab:ab + 384],
                              pg[0:32, 512 * pi:512 * pi + 384],
                              AF.Sigmoid).then_inc(s_act, 1)   # f, i, o
                ac.activation(actsb[0:32, ab + 384:ab + 512],
                              pg[0:32, 512 * pi + 384:512 * pi + 512],
                              AF.Tanh).then_inc(s_act, 1)      # g
                # tanh(c_{t+1})
                ac.wait_ge(s_cp, t + 1)
                if t >= 1:
                    ac.wait_ge(s_hb, t)                        # tcb free
                ac.activation(tcb[0:32, :], csb[0:32, 128 * po:128 * po + 128],
                              AF.Tanh).then_inc(s_tc, 1)
                # h^T: pt[pi] -> hst[pi] (ACT is idle here; frees the DVE)
                ac.wait_ge(s_T, t + 1)
                if t >= 2 and not ABL_NOBCAST:
                    ac.wait_ge(s_hfree, t - 1)                 # hst[pi] flushed
                ac.copy(hst[:, HB * pi:HB * (pi + 1)].bitcast(bf16),
                        pt[:, 512 * pi:512 * pi + 16].bitcast(bf16)
                        ).then_inc(s_hrdy, 1)

        @blk.vector
        def _(ve):
            for t in range(T):
                pi = t % 2
                po = (t + 1) % 2
                ab = 512 * pi
                # c' = f*c + i*g  (batch-major [32,128], all bf16 -> 2x mode)
                ve.wait_ge(s_act, 2 * t + 1)
                ve.tensor_mul(t1b[0:32, :], actsb[0:32, ab:ab + 128],
                              csb[0:32, 128 * pi:128 * pi + 128]).then_inc(s_dvf, 1)
                ve.wait_ge(s_act, 2 * t + 2)
                ve.tensor_mul(t2b[0:32, :], actsb[0:32, ab + 128:ab + 256],
                              actsb[0:32, ab + 384:ab + 512]).then_inc(s_dvf, 1)
                if t >= 2:
                    ve.wait_ge(s_tc, t - 1)                    # csb[po] free
                ve.wait_ge(s_dvf, 2 * t + 2)                   # t1/t2 writeback fence
                ve.tensor_add(csb[0:32, 128 * po:128 * po + 128],
                              t1b[0:32, :], t2b[0:32, :]).then_inc(s_cp, 1)
                # h = o * tanh(c')
                ve.wait_ge(s_tc, t + 1)
                if t >= 2:
                    ve.wait_ge(s_T, t - 1)                     # hsb[pi] free
                ve.tensor_mul(hsb[0:32, 128 * pi:128 * (pi + 1)],
                              actsb[0:32, ab + 256:ab + 384],
                              tcb[0:32, :]).then_inc(s_hb, 1)
                # FC evac psum -> staging ring (output index j = t-1)
                if t >= 1 and not ABL_NOFC:
                    j = t - 1
                    ve.wait_ge(s_fcm, j + 1)
                    if j >= FC_RING:
                        ve.wait_ge(s_fout, 16 * (j // FC_RING))  # ring slot free
                    ve.tensor_copy(fcring[0:32, LOCO * (j % FC_RING):
                                           LOCO * (j % FC_RING + 1)],
                                   pfc[0:32, 512 * (j % 2):
                                       512 * (j % 2) + LOCO]).then_inc(s_fce, 1)
            # epilogue evac for output index T-1
            if not ABL_NOFC:
                j = T - 1
                ve.wait_ge(s_fcm, j + 1)
                ve.wait_ge(s_fout, 16 * (j // FC_RING))
                ve.tensor_copy(fcring[0:32, LOCO * (j % FC_RING):
                                       LOCO * (j % FC_RING + 1)],
                               pfc[0:32, 512 * (j % 2):
                                   512 * (j % 2) + LOCO]).then_inc(s_fce, 1)

        @blk.sync
        def _(sp):
            nxt = 16 * KX * min(XT_AHEAD, T)
            for t in range(T):
                tf = t + XT_AHEAD
                if tf < T and not ABL_NOX:
                    if t >= 2:
                        sp.wait_ge(s_mm, t - 1)                # ring slot free
                    xs = 128 * (tf % XT_RING)
                    for d in range(KX):
                        sp.dma_start(r(xt[:, xs + 32 * d:xs + 32 * (d + 1)]),
                                     r(xT[128 * d:128 * (d + 1), tf, :])).then_inc(
                                         s_xt, 16)
                    nxt += 16 * KX
                    sp.wait_ge(s_xt, nxt)
                    sp.sem_inc(s_xtg, 1)
                # output chunk q covers output indices [8q, 8q+8)
                if t % FC_RING == 0 and t >= FC_RING and not ABL_NOFC:
                    q = t // FC_RING - 1
                    sp.wait_ge(s_fce, FC_RING * (q + 1))
                    sp.dma_start(outF[:, LOCO * FC_RING * q:
                                      LOCO * FC_RING * (q + 1)],
                                 fcring[0:32, :]).then_inc(s_fout, 16)
                    sp.wait_ge(s_fout, 16 * (q + 1))
            # final output chunk (indices [T-8, T))
            if not ABL_NOFC:
                q = T // FC_RING - 1
                sp.wait_ge(s_fce, T)
                sp.dma_start(outF[:, LOCO * FC_RING * q:LOCO * FC_RING * (q + 1)],
                             fcring[0:32, :]).then_inc(s_fout, 16)
                sp.wait_ge(s_fout, 16 * (q + 1))

        @blk.gpsimd
        def _(gp):
            if ABL_NOBCAST:
                return
            from concourse import library_config
            gp.load_library(library_config.remote_dma)
            pidreg = gp.alloc_register("pidreg")
            gp.reg_load(pidreg, nc.partition_id_tensor[0:1, 0:1])
            RD = [(0, k) for k in range(NCORES)]

            def prep(round_t):
                """Queue the round-t broadcast: hst parity round_t%2 into
                slot `pid` of gath parity (round_t+1)%2 on all 8 cores (slot
                and arrival sem chosen by sender rank via the If-chain)."""
                ps = HB * (round_t % 2)
                gbase = 128 * ((round_t + 1) % 2)
                for k in range(NCORES):
                    with gp.If_eq(pidreg, k):
                        gp.remote_dma_broadcast(
                            gath[:, gbase + 16 * k:gbase + 16 * (k + 1)],
                            hst[:, ps:ps + HB],
                            s_harr[k][round_t % 2], s_sent[round_t % 2],
                            rdests=RD).then_inc(s_prep, 1)

            for t in range(T):
                # Mirror the PE's arrival waits (always already satisfied
                # here, transitively through this core's own PE step t) so
                # the round-t send carries "every peer consumed round t-2"
                # explicitly on the sending engine. Must precede the desc-gen
                # (ordering binds at prep time).
                if t >= 1:
                    for k in range(NCORES):
                        gp.wait_ge(*harr_wait(k, t))
                prep(t)
                # Q7 desc-gen must commit to the SBUF ring before the
                # trigger's doorbell write (HW race; the sim models prep
                # synchronously and can't catch it).
                gp.wait_ge(s_prep, t + 1)
                gp.wait_ge(s_hrdy, t + 1)
                gp.trigger_dma(1)
                # Flush-wait for the PREVIOUS round (its packets left during
                # step t's compute, so this is free), then relay to the ACT
                # (hst[(t-1)%2] reusable). Keeps round t's flush latency off
                # the Pool's critical path.
                if t >= 1:
                    gp.wait_ge(s_sent[(t - 1) % 2],
                               SENT_PER_ROUND * ((t + 1) // 2))
                    gp.sem_inc(s_hfree, 1)
            # quiescence: all sends flushed, all peers' packets landed
            gp.wait_ge(s_sent[(T - 1) % 2], SENT_PER_ROUND * ((T + 1) // 2))
            gp.sem_inc(s_hfree, 1)
            for k in range(NCORES):
                gp.wait_ge(s_harr[k][0], 2 * ((T + 1) // 2))
                gp.wait_ge(s_harr[k][1], 2 * (T // 2))

    # ================= Block E: final cleanup =================
    with nc.Block() as blk:
        @blk.gpsimd
        def _(gp):
            for sm in all_sems:
                gp.sem_clear(sm)

    for cm in reversed(ctx_tensors):
        cm.__exit__(None, None, None)
    mybir.codegen_inst_isa_subclasses(nc)
    return nc


def _prep_in_maps(inputs, T=S):
    """Host-side sharding: per-core input dicts (gate order f|i|g|o)."""
    x = np.ascontiguousarray(np.asarray(inputs["x"], np.float32)[:, :T, :])
    W_f = np.asarray(inputs["W_f"], np.float32)
    W_i = np.asarray(inputs["W_i"], np.float32)
    W_g = np.asarray(inputs["W_g"], np.float32)
    W_o = np.asarray(inputs["W_o"], np.float32)
    b_f = np.asarray(inputs["b_f"], np.float32)
    b_i = np.asarray(inputs["b_i"], np.float32)
    b_g = np.asarray(inputs["b_g"], np.float32)
    b_o = np.asarray(inputs["b_o"], np.float32)
    W_fc = np.ascontiguousarray(np.asarray(inputs["W_fc"], np.float32))
    b_fc = np.asarray(inputs["b_fc"], np.float32)

    import ml_dtypes
    bf16 = ml_dtypes.bfloat16
    xT = np.ascontiguousarray(x.transpose(2, 1, 0))  # [DIN, T, B]
    ones = np.ones((1, 256), np.float32)
    eyeb = np.eye(128, dtype=bf16)
    in_maps = []
    for c in range(NCORES):
        sl = slice(LOCH * c, LOCH * (c + 1))
        so = slice(LOCO * c, LOCO * (c + 1))
        Wcat = np.concatenate(
            [W_f[:, sl], W_i[:, sl], W_o[:, sl], W_g[:, sl]], axis=1)  # f|i|o|g
        Wx_c = np.ascontiguousarray(Wcat[:DIN])
        Wh_c = Wcat[DIN:]                                  # [DH, LOCG]
        Wfc_c = W_fc[:, so]
        b_c = np.concatenate([b_f[sl], b_i[sl], b_o[sl], b_g[sl]])[None, :]
        in_maps.append({
            "xT": xT,
            "wxin": Wx_c,
            "whin": np.ascontiguousarray(Wh_c).astype(bf16),
            "wfcin": np.ascontiguousarray(Wfc_c).astype(bf16),
            "bin": np.ascontiguousarray(b_c),
            "bfcin": np.ascontiguousarray(b_fc[None, so]),
            "onesin": ones,
            "identbin": eyeb,
        })
    return in_maps


def _assemble(results, T=S):
    out = np.empty((B, T, DOUT), np.float32)
    for c in range(NCORES):
        blk = np.asarray(results[c]["outF"], np.float32).reshape(B, T, LOCO)
        out[:, :, LOCO * c:LOCO * (c + 1)] = blk
    return out


def get_nc(T=S):
    if T not in _cache:
        _cache[T] = _build_nc(T)
    return _cache[T]


def kernel(**inputs):
    from concourse import bass_utils
    nc = get_nc(S)
    in_maps = _prep_in_maps(inputs, S)
    res = bass_utils.run_bass_kernel_spmd(nc, in_maps, core_ids=list(range(NCORES)))
    return _assemble(res.results, S)
